# revision 6
# baseline (speedup 1.0000x reference)
"""Single-head attention with QKV projections for TRN2, batch-sharded across
8 NeuronCores (one batch element per core).

Reference computation per batch element (S=2048, D=1024, fp32):
    Q = xq @ Wq + bq ; K = xk @ Wk + bk ; V = xv @ Wv + bv
    L = Q @ K^T                      # [S, S]
    out = (softmax(L, -1) * 1/sqrt(D)) @ V

v4: the whole projection/logits pipeline runs on 2-byte PE dtypes. On TRN2,
fp32r matmuls are self-loading (serial ~107ns weight load each; standalone
Ldweights is broken for 4-byte dtypes), while 2-byte matmuls get a bass-
emitted standalone Ldweights pipelined behind the previous matmul. So:
  * W is converted to fp16 on the host; x tiles are DMA'd fp32 and converted
    to fp16 on the (otherwise idle) GpSimd engine before the PE transposes.
  * PE transposes are fp16 (1.0 cyc/row vs 1.5 for f32r, pipelined LDW,
    1-bank fp16 PSUM tiles, half the drain traffic).
  * Projections are fp16 x fp16 -> fp32 PSUM -> fp16 Q^T/K^T resident in
    SBUF [D, S] (32KB/partition each, no DRAM roundtrip) / bf16 V.
  * Logits are fp16 (K^T-tile stationary, Q^T moving straight from SBUF).
  * exp -> bf16 U^T (bf16 range needed: no max subtraction); AV bf16.
  * Rowsum of exp(L^T) comes from 2-column matmuls reusing the AV
    stationaries; reciprocal+scale then normalizes the AV drains.
  * DMA: x tiles ride the sync HWDGE queue (phase-first strips alternate
    both queues), weights ride scalar; each phase prefetches the next
    phase's first x strip.

Precision: fp16 x/W/Q^T/K^T (exact fp16 products, fp32 accumulation) adds
~5e-4 rel err in quadrature with the bf16 AV path; measured ~7e-3 vs the
2e-2 gate.
"""
import numpy as np
from contextlib import ExitStack

import concourse.bass as bass
import concourse.bacc as bacc
import concourse.tile as tile
import concourse.mybir as mybir
from concourse.bass_utils import run_bass_kernel_spmd

F32 = mybir.dt.float32
F32R = mybir.dt.float32r
F16 = mybir.dt.float16
BF16 = mybir.dt.bfloat16
AF = mybir.ActivationFunctionType

B, S, D = 8, 2048, 1024
NKT = D // 128          # 8 contraction tiles
NST = S // 128          # 16 s tiles
SCALE = 1.0 / 32.0      # 1/sqrt(D)

_CACHED = {}


def build(nrep=1, barrier=False):
    nc = bacc.Bacc("TRN2", target_bir_lowering=False, debug=False, num_devices=8)

    xq = nc.dram_tensor("xq", [S, D], F32R, kind="ExternalInput")
    xk = nc.dram_tensor("xk", [S, D], F32R, kind="ExternalInput")
    xv = nc.dram_tensor("xv", [S, D], F32R, kind="ExternalInput")
    wq = nc.dram_tensor("wq", [D, D], F16, kind="ExternalInput")
    wk = nc.dram_tensor("wk", [D, D], F16, kind="ExternalInput")
    wv = nc.dram_tensor("wv", [D, D], F16, kind="ExternalInput")
    bqd = nc.dram_tensor("bqd", [128, NKT], F32, kind="ExternalInput")  # bq.reshape(8,128).T
    bkd = nc.dram_tensor("bkd", [128, NKT], F32, kind="ExternalInput")
    bvd = nc.dram_tensor("bvd", [1, D], F32R, kind="ExternalInput")
    identd = nc.dram_tensor("identd", [128, 128], F16, kind="ExternalInput")
    ones1d = nc.dram_tensor("ones1d", [1, 128], F32R, kind="ExternalInput")
    onespd = nc.dram_tensor("onespd", [128, 2], BF16, kind="ExternalInput")

    out = nc.dram_tensor("out", [S, D], F32, kind="ExternalOutput")

    with tile.TileContext(nc) as tc, ExitStack() as ctx:
        # ---------------- persistent pools ----------------
        cpool = ctx.enter_context(tc.tile_pool(name="const", bufs=1))
        qtp = ctx.enter_context(tc.tile_pool(name="qtr", bufs=1))
        ktp = ctx.enter_context(tc.tile_pool(name="ktr", bufs=1))
        vsp = ctx.enter_context(tc.tile_pool(name="vres", bufs=1))
        pp = ctx.enter_context(tc.tile_pool(name="pp", bufs=2, space="PSUM"))

        ident = cpool.tile([128, 128], F16, tag="ident")
        bqs = cpool.tile([128, NKT], F32, tag="bqs")
        bks = cpool.tile([128, NKT], F32, tag="bks")
        ones1 = cpool.tile([1, 128], F32R, tag="ones1")
        onesp = cpool.tile([128, 2], BF16, tag="onesp")
        bvb = cpool.tile([128, D], F32, tag="bvb")
        nc.sync.dma_start(ident[:], identd.ap())
        nc.gpsimd.dma_start(bqs[:], bqd.ap())
        nc.gpsimd.dma_start(bks[:], bkd.ap())
        nc.gpsimd.dma_start(ones1[:], ones1d.ap())
        nc.gpsimd.dma_start(onesp[:], onespd.ap())

        def broadcast_bv(bvctx):
            # bvb = ones1.T @ bvs via a K=1 matmul (issued at phase-B entry so
            # it does not block the PE queue head at startup)
            bvsp = bvctx.enter_context(tc.tile_pool(name="bvsp", bufs=1))
            bvs = bvsp.tile([1, D], F32R, tag="bvs")
            nc.gpsimd.dma_start(bvs[:], bvd.ap())
            for h in range(2):
                bps = pp.tile([128, 1024], F32, tag="pp")
                nc.tensor.matmul(bps[:, 0:512], ones1[:], bvs[:, h * 512:(h + 1) * 512],
                                 start=True, stop=True)
                nc.scalar.copy(bvb[:, h * 512:(h + 1) * 512], bps[:, 0:512])

        # ---------------- phase A: projections ----------------
        def load_w(wpool, w_dram):
            # weights ride the scalar HWDGE queue only - a weight load blocked
            # on the previous phase's buffer must not block x tiles
            w_s = wpool.tile([128, NKT * D], F16, tag="w")
            for k in range(NKT):
                nc.scalar.dma_start(w_s[:, k * D:(k + 1) * D],
                                    w_dram.ap()[k * 128:(k + 1) * 128, :])
            return w_s

        def load_x_strip(xpool, xhpool, x_dram, j, n_stiles, alternate=False):
            """DMA x rows [j*128*n ..) as fp32, then GpSimd-convert each tile
            to fp16. Phase-first strips alternate both HWDGE queues; later
            strips ride sync only (scalar belongs to weights)."""
            xhs = []
            for st in range(n_stiles):
                xl = xpool.tile([128, D], F32R, tag="xl")
                dma = nc.scalar if (alternate and st % 2 == 1) else nc.sync
                dma.dma_start(
                    xl[:], x_dram.ap()[(j * n_stiles + st) * 128:(j * n_stiles + st + 1) * 128, :])
                xh = xhpool.tile([128, D], F16, tag="xh")
                nc.gpsimd.tensor_copy(xh[:], xl[:])
                xhs.append(xh)
            return xhs

        def transpose_strip(tp, xtpool, xhs, n_stiles):
            """PE-transpose fp16 x tiles into an x^T strip [D, 128*n]
            (fp16, k-major: xt[:, k*128*n + st*128 + c]). One 1-bank fp16 PSUM
            tile per s-tile, drained by one wide strided scatter."""
            xt = xtpool.tile([128, NKT * 128 * n_stiles], F16, tag="xt")
            for st, xh in enumerate(xhs):
                tpt = tp.tile([128, NKT * 128], F16, tag="tp")
                for k in range(NKT):
                    nc.tensor.transpose(tpt[:, k * 128:(k + 1) * 128],
                                        xh[:, k * 128:(k + 1) * 128], ident[:])
                dst = xt[:].rearrange("p (k s) -> p k s", s=128 * n_stiles)
                if st % 2 == 0:
                    nc.vector.tensor_copy(
                        dst[:, :, st * 128:(st + 1) * 128], tpt[:])
                else:
                    nc.scalar.copy(
                        dst[:, :, st * 128:(st + 1) * 128], tpt[:])
            return xt

        for _rep in range(nrep):
          if _rep and barrier:
              tc.strict_bb_all_engine_barrier()
          with ExitStack() as rctx:
            qt = qtp.tile([128, NKT * S], F16, tag="qt")     # Q^T resident fp16
            kt = ktp.tile([128, NKT * S], F16, tag="kt")     # K^T resident fp16
            vs = vsp.tile([128, NST * D], BF16, tag="vs")    # V resident bf16

            with ExitStack() as actx:
              tp = actx.enter_context(tc.tile_pool(name="tp", bufs=3, space="PSUM"))
              wpool = actx.enter_context(tc.tile_pool(name="wpool", bufs=1))
              xpool = actx.enter_context(tc.tile_pool(name="xpool", bufs=6))
              xhpool = actx.enter_context(tc.tile_pool(name="xhpool", bufs=10))
              xtpool = actx.enter_context(tc.tile_pool(name="xtpool", bufs=1))

              def proj_qk(w_s, dst, bias, xls0, next_x, next_n):
                  """Q^T/K^T projection -> dst [d_out-major, s] fp16; returns
                  the prefetched first strip of the next phase's x."""
                  nxt = None
                  for j in range(2):
                      xls = xls0 if j == 0 else load_x_strip(xpool, xhpool, x_cur, j, 8)
                      xt = transpose_strip(tp, xtpool, xls, 8)
                      if j == 1:
                          nxt = load_x_strip(xpool, xhpool, next_x, 0, next_n,
                                             alternate=True)
                      for m in range(NKT):
                          ppt = pp.tile([128, 1024], F32, tag="pp")
                          for k in range(NKT):
                              for h in range(2):
                                  nc.tensor.matmul(
                                      ppt[:, h * 512:(h + 1) * 512],
                                      w_s[:, k * D + m * 128:k * D + (m + 1) * 128],
                                      xt[:, k * 1024 + h * 512:k * 1024 + (h + 1) * 512],
                                      start=(k == 0), stop=(k == NKT - 1))
                          nc.scalar.activation(
                              dst[:, m * S + j * 1024:m * S + (j + 1) * 1024],
                              ppt[:], AF.Identity, bias=bias[:, m:m + 1])
                  return nxt

              # ---- A-Q: Q^T resident fp16 ----
              with nc.named_scope("phase_aq"):
                  x_cur = xq
                  xls_q = load_x_strip(xpool, xhpool, xq, 0, 8, alternate=True)
                  w_s = load_w(wpool, wq)
                  xls_k = proj_qk(w_s, qt, bqs, xls_q, xk, 8)

              # ---- A-K: K^T resident fp16 ----
              with nc.named_scope("phase_ak"):
                  x_cur = xk
                  w_s = load_w(wpool, wk)
                  xls_v = proj_qk(w_s, kt, bks, xls_k, xv, 4)

              # ---- A-V: V resident bf16 ----
              with nc.named_scope("phase_av"):
                w_s = load_w(wpool, wv)
                for j in range(4):
                    xls = xls_v if j == 0 else load_x_strip(xpool, xhpool, xv, j, 4)
                    xt = transpose_strip(tp, xtpool, xls, 4)
                    for m in range(4):          # s tiles within strip
                        sg = j * 4 + m
                        ppt = pp.tile([128, 1024], F32, tag="pp")
                        for k in range(NKT):
                            for h in range(2):
                                nc.tensor.matmul(
                                    ppt[:, h * 512:(h + 1) * 512],
                                    xt[:, k * 512 + m * 128:k * 512 + (m + 1) * 128],
                                    w_s[:, k * D + h * 512:k * D + (h + 1) * 512],
                                    start=(k == 0), stop=(k == NKT - 1))
                        nc.scalar.copy(vs[:, sg * D:(sg + 1) * D], ppt[:])

            # ---------------- phase B: attention (transposed logits) ----------------
            with ExitStack() as bctx, nc.named_scope("phase_b"):
              op = bctx.enter_context(tc.tile_pool(name="op", bufs=2, space="PSUM"))
              utp = bctx.enter_context(tc.tile_pool(name="utp", bufs=2))
              osp = bctx.enter_context(tc.tile_pool(name="osp", bufs=2))
              rsp = bctx.enter_context(tc.tile_pool(name="rsp", bufs=2))
              rsps = bctx.enter_context(tc.tile_pool(name="rsps", bufs=2, space="PSUM"))

              if _rep == 0:
                  broadcast_bv(bctx)

              for j in range(2):                  # q strips of 1024
                # L^T tiles + exp -> U^T strip [S, 1024] (bf16)
                ut = utp.tile([128, NST * 1024], BF16, tag="ut")
                for t in range(NST):
                    lpt = pp.tile([128, 1024], F32, tag="pp")
                    for k in range(NKT):
                        for h in range(2):
                            nc.tensor.matmul(
                                lpt[:, h * 512:(h + 1) * 512],
                                kt[:, k * S + t * 128:k * S + (t + 1) * 128],
                                qt[:, k * S + j * 1024 + h * 512:k * S + j * 1024 + (h + 1) * 512],
                                start=(k == 0), stop=(k == NKT - 1))
                    nc.scalar.activation(ut[:, t * 1024:(t + 1) * 1024],
                                         lpt[:], AF.Exp)

                for m in range(8):              # q tiles of 128 within strip
                    sq = j * 8 + m
                    # rowsum via 2-col matmuls sharing the AV stationaries
                    rs = rsps.tile([128, 2], F32, tag="rs")
                    rct = rsp.tile([128, 1], F32, tag="rct")
                    os_t = osp.tile([128, D], F32, tag="os")
                    for h in range(2):
                        opt = op.tile([128, 512], F32, tag="av")
                        for t in range(NST):
                            st_ap = ut[:, t * 1024 + m * 128:t * 1024 + (m + 1) * 128]
                            nc.tensor.matmul(
                                opt[:],
                                st_ap,
                                vs[:, t * D + h * 512:t * D + (h + 1) * 512],
                                start=(t == 0), stop=(t == NST - 1))
                            if h == 0:
                                nc.tensor.matmul(
                                    rs[:], st_ap, onesp[:],
                                    start=(t == 0), stop=(t == NST - 1))
                        if h == 0:
                            nc.vector.reciprocal(rct[:], rs[:, 0:1])
                            nc.vector.tensor_scalar_mul(rct[:], rct[:], SCALE)
                        nc.vector.tensor_scalar_mul(
                            os_t[:, h * 512:(h + 1) * 512], opt[:], rct[:])
                        nc.vector.tensor_add(
                            os_t[:, h * 512:(h + 1) * 512],
                            os_t[:, h * 512:(h + 1) * 512],
                            bvb[:, h * 512:(h + 1) * 512])
                        nc.scalar.dma_start(
                            out.ap()[sq * 128:(sq + 1) * 128, h * 512:(h + 1) * 512],
                            os_t[:, h * 512:(h + 1) * 512])

    nc.compile()
    return nc


def _get_nc():
    if "nc" not in _CACHED:
        _CACHED["nc"] = build()
    return _CACHED["nc"]


def _bf16_ones(shape):
    import ml_dtypes
    return np.ones(shape, ml_dtypes.bfloat16)


def make_in_maps(q, k, v, Wq, bq, Wk, bk, Wv, bv):
    q = np.ascontiguousarray(q, np.float32)
    k = np.ascontiguousarray(k, np.float32)
    v = np.ascontiguousarray(v, np.float32)
    consts = {
        "wq": np.ascontiguousarray(np.asarray(Wq, np.float32).astype(np.float16)),
        "wk": np.ascontiguousarray(np.asarray(Wk, np.float32).astype(np.float16)),
        "wv": np.ascontiguousarray(np.asarray(Wv, np.float32).astype(np.float16)),
        "bqd": np.ascontiguousarray(np.asarray(bq, np.float32).reshape(NKT, 128).T),
        "bkd": np.ascontiguousarray(np.asarray(bk, np.float32).reshape(NKT, 128).T),
        "bvd": np.asarray(bv, np.float32).reshape(1, D).copy(),
        "identd": np.eye(128, dtype=np.float16),
        "ones1d": np.ones((1, 128), np.float32),
        "onespd": _bf16_ones((128, 2)),
    }
    return [dict(consts, xq=q[c], xk=k[c], xv=v[c]) for c in range(B)]


def kernel(q, k, v, Wq, bq, Wk, bk, Wv, bv, _trace=False, _trace_kwargs=None):
    in_maps = make_in_maps(q, k, v, Wq, bq, Wk, bk, Wv, bv)
    nc = _get_nc()
    res = run_bass_kernel_spmd(nc, in_maps, core_ids=list(range(B)),
                               trace=_trace, **(_trace_kwargs or {}))
    out = np.stack([res.results[c]["out"] for c in range(B)])
    if _trace:
        kernel.last_results = res
    return out
